# revision 32
# baseline (speedup 1.0000x reference)
"""Distributed Trainium2 kernel for causal multi-head attention with RoPE.

Problem (hardcoded): B=2, S=2048, D=2048, H=16, DH=128, float32 I/O.
  out = softmax(mask + rope(x@wq.T) @ rope(x@wk.T).T / sqrt(DH)) @ (x@wv.T) @ wo.T

Sharding over 8 NeuronCores: batch (2) x head-group (4).
Core c handles batch b=c//4 and heads [4g, 4g+4) with g=c%4:
  - QKV projections computed in transposed layout qT/kT [d, tok] (bf16 compute,
    f32 accumulation in PSUM); v in [tok, d] layout.
  - RoPE applied in transposed layout: rot = qT*C + pairswap(qT)*S, where the
    pair swap runs on the vector engine (stream_shuffle within 32-partition
    quadrants) and C/S are host-built [128, 2048] matrices from freqs_cos/sin.
    1/sqrt(DH) is folded into wq. The final add runs on GpSimd.
  - Causal attention per head in transposed score layout [k, q]: masked exp
    tiles feed both attn@V and a ones-row matmul that accumulates the softmax
    denominators (no max-subtraction: scores are O(3)). Normalization
    multiplies by a PE-broadcast-free reciprocal of the replicated row sums.
  - Per-head 8-way AllToAll ships each head's normalized output to both
    batch-candidate peers; the sender zeroes the wrong-batch copy (avl/avh on
    DVE/GpSimd), so receivers just add the two halves -- no runtime select.
    A warm-up collective at t=0 absorbs the ~5-40us cross-core launch skew
    during the DMA-bound input load, so the first real collective does not
    stall the wo chains. The wo rows and a2a results for head h are fetched
    right after collective h so the DMA queues stay clear for later sends.
  - Output projection is token-parallel: each core computes its 512 tokens for
    all 2048 output columns with the full wo. Heads 0-2 accumulate into fp16
    SBUF partials (osb); only a 4-matmul tail per chain waits on the final
    collective.
Host: shards/prepares inputs per core, runs one SPMD NEFF on cores 0-7,
assembles out[b, 512g:512(g+1), :] from each core (fp16 device output).
"""

import sys

for _p in ("/opt/trn_rl_repo", "/root/.axon_site/_ro/trn_rl_repo"):
    if _p not in sys.path:
        sys.path.insert(0, _p)

import math
import numpy as np
import ml_dtypes

import concourse.bass as bass
import concourse.bacc as bacc
import concourse.mybir as mybir
from concourse import tile
from concourse.bass_utils import run_bass_kernel_spmd

bf16 = ml_dtypes.bfloat16
F32 = mybir.dt.float32
F16 = mybir.dt.float16
BF16 = mybir.dt.bfloat16
Exp = mybir.ActivationFunctionType.Exp

B, S, D, H = 2, 2048, 2048, 16
DH = D // H  # 128
HPC = 4  # heads per core
GROUPS = [[0, 1, 2, 3, 4, 5, 6, 7]]
NIC = D // 128  # 16 contraction chunks
NTB = S // 512  # 4 token blocks of 512
NTC = S // 128  # 16 token chunks of 128
SWAP_MASK = [i ^ 1 for i in range(32)]  # pair swap within 32-partition quads

_GRAPH_CACHE = {}


def build_graph():
    if "nc" in _GRAPH_CACHE:
        return _GRAPH_CACHE["nc"]
    nc = bacc.Bacc(None)

    xT_d = nc.declare_dram_parameter("xT", [D, S], BF16, isOutput=False)
    wqT_d = nc.declare_dram_parameter("wqT", [D, 512], BF16, isOutput=False)
    wkT_d = nc.declare_dram_parameter("wkT", [D, 512], BF16, isOutput=False)
    wvT_d = nc.declare_dram_parameter("wvT", [D, 512], BF16, isOutput=False)
    woT_d = nc.declare_dram_parameter("woT", [D, D], BF16, isOutput=False)
    cmat_d = nc.declare_dram_parameter("cmat", [128, S], F32, isOutput=False)
    smat_d = nc.declare_dram_parameter("smat", [128, S], F32, isOutput=False)
    mmul_d = nc.declare_dram_parameter("mmul", [128, 512], BF16, isOutput=False)
    gsel_d = nc.declare_dram_parameter("gsel", [128, 2], F32, isOutput=False)
    out_d = nc.declare_dram_parameter("out", [512, D], F16, isOutput=True)

    a2a_in = [nc.dram_tensor(f"a2a_in{h}", [1024, 512], BF16) for h in range(HPC)]
    a2a_out = [nc.dram_tensor(f"a2a_out{h}", [1024, 512], BF16) for h in range(HPC)]
    warm_in = nc.dram_tensor("warm_in", [8, 16], BF16)
    warm_out = nc.dram_tensor("warm_out", [8, 16], BF16)

    with tile.TileContext(nc) as tc:
        with tc.tile_pool(name="work", bufs=3) as wk:
            with tc.tile_pool(name="poolA", bufs=1) as pa:
                # persistent across QKV + attention
                mmul_sb = pa.tile([128, 512], BF16, tag="mmul")
                gsel_sb = pa.tile([128, 2], F32, tag="gsel")
                nc.sync.dma_start(gsel_sb[:], gsel_d[:])
                ones_mat = pa.tile([128, 128], BF16, tag="ones_mat")
                nc.vector.memset(ones_mat[:], 1.0)
                qrot = [pa.tile([128, S], BF16, tag=f"q{h}", name=f"qrot{h}") for h in range(HPC)]
                krot = [pa.tile([128, S], BF16, tag=f"k{h}", name=f"krot{h}") for h in range(HPC)]
                vsb = [pa.tile([128, 512], BF16, tag=f"v{j}", name=f"vsb{j}") for j in range(NTC)]

                # ============ Stage 1+2: QKV projections + RoPE =============
                with (
                    tc.tile_pool(name="qkvw", bufs=1) as qw,
                    tc.tile_pool(name="psq", bufs=5, space="PSUM") as psq,
                    tc.tile_pool(name="psv", bufs=2, space="PSUM") as psv,
                ):
                    xt = [qw.tile([128, S], BF16, tag=f"xt{i}", name=f"xt{i}") for i in range(NIC)]
                    wq_sb = [qw.tile([128, 512], BF16, tag=f"wq{i}", name=f"wqsb{i}") for i in range(NIC)]
                    wk_sb = [qw.tile([128, 512], BF16, tag=f"wk{i}", name=f"wksb{i}") for i in range(NIC)]
                    wv_sb = [qw.tile([128, 512], BF16, tag=f"wv{i}", name=f"wvsb{i}") for i in range(NIC)]
                    cs_sb = qw.tile([128, S], F32, tag="cs")
                    sn_sb = qw.tile([128, S], F32, tag="sn")
                    # DMA priority: x + wq stream first (gates first head's
                    # matmuls), then rope tables, then wk, wv, mask.
                    for i in range(NIC):
                        nc.sync.dma_start(xt[i][:], xT_d[128 * i : 128 * (i + 1), :])
                        nc.sync.dma_start(wq_sb[i][:], wqT_d[128 * i : 128 * (i + 1), :])
                    nc.sync.dma_start(cs_sb[:], cmat_d[:])
                    nc.sync.dma_start(sn_sb[:], smat_d[:])
                    for i in range(NIC):
                        nc.sync.dma_start(wk_sb[i][:], wkT_d[128 * i : 128 * (i + 1), :])
                    for i in range(NIC):
                        nc.sync.dma_start(wv_sb[i][:], wvT_d[128 * i : 128 * (i + 1), :])
                    nc.sync.dma_start(mmul_sb[:], mmul_d[:])

                    # Q and K projections -> transposed layout [d, tok] + RoPE.
                    # Each pass runs one head's 4 block-chains interleaved
                    # chunk-wise so every arriving x chunk feeds all of them.
                    def qk_pass(specs):
                        pss = [psq.tile([128, 512], F32, tag="qk", name=f"qk{ci}") for ci in range(len(specs))]
                        for i in range(NIC):
                            for ci, (w_sb, rot, h, b) in enumerate(specs):
                                nc.tensor.matmul(
                                    pss[ci][:],
                                    w_sb[i][:, 128 * h : 128 * (h + 1)],
                                    xt[i][:, 512 * b : 512 * (b + 1)],
                                    start=(i == 0),
                                    stop=(i == NIC - 1),
                                )
                        for ci, (w_sb, rot, h, b) in enumerate(specs):
                            ps = pss[ci]
                            shp = wk.tile([128, 512], F32, tag="shp")
                            nc.vector.stream_shuffle(shp[:], ps[:], SWAP_MASK)
                            t1 = wk.tile([128, 512], F32, tag="t1")
                            t2 = wk.tile([128, 512], F32, tag="t2")
                            nc.vector.tensor_mul(t1[:], ps[:], cs_sb[:, 512 * b : 512 * (b + 1)])
                            nc.vector.tensor_mul(t2[:], shp[:], sn_sb[:, 512 * b : 512 * (b + 1)])
                            nc.gpsimd.tensor_add(rot[h][:, 512 * b : 512 * (b + 1)], t1[:], t2[:])

                    for w_sb, rot in ((wq_sb, qrot), (wk_sb, krot)):
                        for h in range(HPC):
                            qk_pass([(w_sb, rot, h, b) for b in range(NTB)])

                    # V projection -> [tok, d] layout
                    for j in range(NTC):
                        ps = psv.tile([128, 512], F32, tag="v")
                        for i in range(NIC):
                            nc.tensor.matmul(
                                ps[:],
                                xt[i][:, 128 * j : 128 * (j + 1)],
                                wv_sb[i][:],
                                start=(i == 0),
                                stop=(i == NIC - 1),
                            )
                        nc.scalar.copy(vsb[j][:], ps[:])

                # Warm the CC path only now: any collective activity drops PE
                # matmul throughput ~18% for the rest of the kernel, so the
                # QKV phase above must run before the first collective. The
                # input copy from vsb pins the dispatch after the projections.
                nc.sync.dma_start(warm_in[:], vsb[NTC - 1][0:8, 0:16])
                nc.gpsimd.collective_compute(
                    "AllToAll",
                    mybir.AluOpType.bypass,
                    replica_groups=GROUPS,
                    ins=[warm_in[:]],
                    outs=[warm_out[:]],
                )

                with tc.tile_pool(name="wosb", bufs=1) as wop:
                    wo_sb = [wop.tile([128, D], BF16, tag=f"wo{cc}", name=f"wosb{cc}") for cc in range(NIC)]

                    # ============ Stage 3: attention per head ===============
                    with (
                        tc.tile_pool(name="attn", bufs=3) as at,
                        tc.tile_pool(name="agp", bufs=1) as agp,
                        tc.tile_pool(name="psb", bufs=3, space="PSUM") as psb,
                        tc.tile_pool(name="psav", bufs=2, space="PSUM") as psav,
                        tc.tile_pool(name="psrs", bufs=1, space="PSUM") as psrs,
                        tc.tile_pool(name="pswo", bufs=2, space="PSUM") as pswo,
                    ):
                        aglo = [agp.tile([128, 512], BF16, tag=f"aglo{g}", name=f"aglo{g}") for g in range(NIC)]
                        aghi = [agp.tile([128, 512], BF16, tag=f"aghi{g}", name=f"aghi{g}") for g in range(NIC)]
                        agc = aglo  # combined in place
                        for h in range(HPC):
                            for b in range(NTB):
                                q0 = 512 * b
                                nk2 = 4 * (b + 1)
                                av = psav.tile([128, 512], F32, tag="av")
                                rsum = psrs.tile([128, 512], F32, tag="rs")
                                for kc in range(nk2):
                                    j = kc - 4 * b  # >= 0 on the diagonal band
                                    o = 128 * j if j > 0 else 0
                                    w = 512 - o
                                    ps = psb.tile([128, 512], F32, tag="sb")
                                    nc.tensor.matmul(
                                        ps[:, :w],
                                        krot[h][:, 128 * kc : 128 * (kc + 1)],
                                        qrot[h][:, q0 + o : q0 + 512],
                                    )
                                    et = at.tile([128, 512], BF16, tag="et", bufs=6)
                                    nc.scalar.activation(et[:, :w], ps[:, :w], Exp)
                                    if j >= 0:
                                        # only the first 128 cols of a diagonal
                                        # tile contain masked elements
                                        nc.vector.tensor_mul(et[:, :128], et[:, :128], mmul_sb[:, :128])
                                    nc.tensor.matmul(
                                        av[:, o:512],
                                        vsb[kc][:, 128 * h : 128 * (h + 1)],
                                        et[:, :w],
                                        start=(kc == 0),
                                        stop=(kc == nk2 - 1),
                                    )
                                    nc.tensor.matmul(
                                        rsum[:, o:512],
                                        ones_mat[:],
                                        et[:, :w],
                                        start=(kc == 0),
                                        stop=(kc == nk2 - 1),
                                    )
                                # denominators arrive replicated across partitions
                                rbc = wk.tile([128, 512], F32, tag="rbc")
                                nc.vector.reciprocal_approx_fast(out=rbc[:], in_=rsum[:])
                                avn = at.tile([128, 512], BF16, tag="avn", bufs=6)
                                nc.vector.tensor_mul(avn[:], av[:], rbc[:])
                                # sender-side batch masking: lo slots carry data
                                # iff this core is batch 0, hi slots iff batch 1
                                avl = at.tile([128, 512], BF16, tag="avl", bufs=4)
                                avh = at.tile([128, 512], BF16, tag="avh", bufs=4)
                                nc.vector.tensor_scalar_mul(avl[:], avn[:], gsel_sb[:, 0:1])
                                nc.gpsimd.tensor_sub(avh[:], avn[:], avl[:])
                                nc.sync.dma_start(a2a_in[h][128 * b : 128 * (b + 1), :], avl[:])
                                nc.sync.dma_start(a2a_in[h][512 + 128 * b : 512 + 128 * (b + 1), :], avh[:])
                            nc.gpsimd.collective_compute(
                                "AllToAll",
                                mybir.AluOpType.bypass,
                                replica_groups=GROUPS,
                                ins=[a2a_in[h][:]],
                                outs=[a2a_out[h][:]],
                            )
                            # pull this head's chunks for all 4 peer head-groups
                            # and this head's wo rows (issued here, not up
                            # front, to keep DMA queues clear for a2a writes)
                            for r in range(4):
                                g = 4 * r + h
                                nc.sync.dma_start(aglo[g][:], a2a_out[h][128 * r : 128 * (r + 1), :])
                                nc.sync.dma_start(aghi[g][:], a2a_out[h][512 + 128 * r : 512 + 128 * (r + 1), :])
                                nc.sync.dma_start(wo_sb[g][:], woT_d[128 * g : 128 * (g + 1), :])

                        # ===== Stage 4: token-parallel wo projection ========
                        # gi order is head-major: heads 0-2 (gi 0-11) accumulate
                        # into SBUF partials while collective 3 is in flight;
                        # only the 4-matmul tail per chain needs head 3.
                        G_ORDER = [4 * r + hh for hh in range(4) for r in range(4)]
                        for g in G_ORDER:
                            nc.gpsimd.tensor_add(aglo[g][:], aglo[g][:], aghi[g][:])
                        osb = [agp.tile([128, D], F16, tag=f"osb{t}", name=f"osb{t}") for t in range(4)]
                        for t in range(4):
                            for op in range(2):
                                pss = [pswo.tile([128, 512], F32, tag="wo", name=f"wops{p}") for p in range(2)]
                                for gi in range(12):
                                    g = G_ORDER[gi]
                                    for p in range(2):
                                        oc = 2 * op + p
                                        nc.tensor.matmul(
                                            pss[p][:],
                                            agc[g][:, 128 * t : 128 * (t + 1)],
                                            wo_sb[g][:, 512 * oc : 512 * (oc + 1)],
                                            start=(gi == 0),
                                            stop=(gi == 11),
                                        )
                                for p in range(2):
                                    oc = 2 * op + p
                                    nc.scalar.copy(osb[t][:, 512 * oc : 512 * (oc + 1)], pss[p][:])
                        for t in range(4):
                            for op in range(2):
                                pss = [pswo.tile([128, 512], F32, tag="wo", name=f"wopsB{p}") for p in range(2)]
                                for gi in range(12, 16):
                                    g = G_ORDER[gi]
                                    for p in range(2):
                                        oc = 2 * op + p
                                        nc.tensor.matmul(
                                            pss[p][:],
                                            agc[g][:, 128 * t : 128 * (t + 1)],
                                            wo_sb[g][:, 512 * oc : 512 * (oc + 1)],
                                            start=(gi == 12),
                                            stop=(gi == 15),
                                        )
                                for p in range(2):
                                    oc = 2 * op + p
                                    nc.vector.tensor_add(
                                        osb[t][:, 512 * oc : 512 * (oc + 1)],
                                        pss[p][:],
                                        osb[t][:, 512 * oc : 512 * (oc + 1)],
                                    )
                            nc.sync.dma_start(out_d[128 * t : 128 * (t + 1), :], osb[t][:])

    nc.finalize()
    _GRAPH_CACHE["nc"] = nc
    return nc


def _host_prep(x, freqs_cos, freqs_sin, wq, wk, wv, wo):
    """Build the 8 per-core input maps."""
    fc = np.asarray(freqs_cos, np.float32)  # [S, 64]
    fs = np.asarray(freqs_sin, np.float32)
    cmat = np.empty((128, S), np.float32)
    smat = np.empty((128, S), np.float32)
    cmat[0::2, :] = fc.T[:, :]  # row 2i   <- cos[:, i]
    cmat[1::2, :] = fc.T[:, :]
    smat[0::2, :] = -fs.T[:, :]  # rot[2i]   = a*c - b*s ; shuf[2i]   = b
    smat[1::2, :] = fs.T[:, :]  # rot[2i+1] = b*c + a*s ; shuf[2i+1] = a

    xs = np.arange(128)[:, None]
    ys = np.arange(512)[None, :]
    # AV-path mask for [128 k x 512 q] diagonal tiles: valid iff x <= y
    mmul = (xs <= ys).astype(np.float32)

    wq_s = np.asarray(wq, np.float32) / math.sqrt(DH)
    wk_s = np.asarray(wk, np.float32)
    wv_s = np.asarray(wv, np.float32)
    woT = np.ascontiguousarray(np.asarray(wo, np.float32).T).astype(bf16)
    x = np.asarray(x, np.float32)

    shared = {
        "cmat": cmat,
        "smat": smat,
        "mmul": mmul.astype(bf16),
        "woT": woT,
    }
    in_maps = []
    for c in range(8):
        b, g = c // 4, c % 4
        hs = slice(512 * g, 512 * (g + 1))
        m = dict(shared)
        m["xT"] = np.ascontiguousarray(x[b].T).astype(bf16)
        m["wqT"] = np.ascontiguousarray(wq_s[hs, :].T).astype(bf16)
        m["wkT"] = np.ascontiguousarray(wk_s[hs, :].T).astype(bf16)
        m["wvT"] = np.ascontiguousarray(wv_s[hs, :].T).astype(bf16)
        gsel = np.zeros((128, 2), np.float32)
        gsel[:, b] = 1.0
        m["gsel"] = gsel
        in_maps.append(m)
    return in_maps


def kernel(x, freqs_cos, freqs_sin, mask, wq, wk, wv, wo):
    in_maps = _host_prep(x, freqs_cos, freqs_sin, wq, wk, wv, wo)
    nc = build_graph()
    results = run_bass_kernel_spmd(nc, in_maps, core_ids=list(range(8))).results
    out = np.empty((B, S, D), np.float32)
    for c in range(8):
        b, g = c // 4, c % 4
        out[b, 512 * g : 512 * (g + 1), :] = results[c]["out"]
    return out


# revision 33
# speedup vs baseline: 1.0160x; 1.0160x over previous
"""Distributed Trainium2 kernel for causal multi-head attention with RoPE.

Problem (hardcoded): B=2, S=2048, D=2048, H=16, DH=128, float32 I/O.
  out = softmax(mask + rope(x@wq.T) @ rope(x@wk.T).T / sqrt(DH)) @ (x@wv.T) @ wo.T

Sharding over 8 NeuronCores: batch (2) x head-group (4).
Core c handles batch b=c//4 and heads [4g, 4g+4) with g=c%4:
  - QKV projections computed in transposed layout qT/kT [d, tok] (bf16 compute,
    f32 accumulation in PSUM); v in [tok, d] layout.
  - RoPE applied in transposed layout: rot = qT*C + pairswap(qT)*S, where the
    pair swap runs on the vector engine (stream_shuffle within 32-partition
    quadrants) and C/S are host-built [128, 2048] matrices from freqs_cos/sin.
    1/sqrt(DH) is folded into wq. The final add runs on GpSimd.
  - Causal attention per head in transposed score layout [k, q]: masked exp
    tiles feed both attn@V and a ones-row matmul that accumulates the softmax
    denominators (no max-subtraction: scores are O(3)). Normalization
    multiplies by a PE-broadcast-free reciprocal of the replicated row sums.
  - Per-head 8-way AllToAll ships each head's normalized output to both
    batch-candidate peers; the sender zeroes the wrong-batch copy (avl/avh on
    DVE/GpSimd), so receivers just add the two halves -- no runtime select.
    A warm-up collective at t=0 absorbs the ~5-40us cross-core launch skew
    during the DMA-bound input load, so the first real collective does not
    stall the wo chains. The wo rows and a2a results for head h are fetched
    right after collective h so the DMA queues stay clear for later sends.
  - Output projection is token-parallel: each core computes its 512 tokens for
    all 2048 output columns with the full wo. Heads 0-2 accumulate into fp16
    SBUF partials (osb); only a 4-matmul tail per chain waits on the final
    collective.
Host: shards/prepares inputs per core, runs one SPMD NEFF on cores 0-7,
assembles out[b, 512g:512(g+1), :] from each core (fp16 device output).
"""

import sys

for _p in ("/opt/trn_rl_repo", "/root/.axon_site/_ro/trn_rl_repo"):
    if _p not in sys.path:
        sys.path.insert(0, _p)

import math
import numpy as np
import ml_dtypes

import concourse.bass as bass
import concourse.bacc as bacc
import concourse.mybir as mybir
from concourse import tile
from concourse.bass_utils import run_bass_kernel_spmd

bf16 = ml_dtypes.bfloat16
F32 = mybir.dt.float32
F16 = mybir.dt.float16
BF16 = mybir.dt.bfloat16
Exp = mybir.ActivationFunctionType.Exp

B, S, D, H = 2, 2048, 2048, 16
DH = D // H  # 128
HPC = 4  # heads per core
GROUPS = [[0, 1, 2, 3, 4, 5, 6, 7]]
NIC = D // 128  # 16 contraction chunks
NTB = S // 512  # 4 token blocks of 512
NTC = S // 128  # 16 token chunks of 128
SWAP_MASK = [i ^ 1 for i in range(32)]  # pair swap within 32-partition quads

_GRAPH_CACHE = {}


def build_graph():
    if "nc" in _GRAPH_CACHE:
        return _GRAPH_CACHE["nc"]
    nc = bacc.Bacc(None)

    xT_d = nc.declare_dram_parameter("xT", [D, S], BF16, isOutput=False)
    wqT_d = nc.declare_dram_parameter("wqT", [D, 512], BF16, isOutput=False)
    wkT_d = nc.declare_dram_parameter("wkT", [D, 512], BF16, isOutput=False)
    wvT_d = nc.declare_dram_parameter("wvT", [D, 512], BF16, isOutput=False)
    woT_d = nc.declare_dram_parameter("woT", [D, D], BF16, isOutput=False)
    cmat_d = nc.declare_dram_parameter("cmat", [128, S], F32, isOutput=False)
    smat_d = nc.declare_dram_parameter("smat", [128, S], F32, isOutput=False)
    mmul_d = nc.declare_dram_parameter("mmul", [128, 512], BF16, isOutput=False)
    gsel_d = nc.declare_dram_parameter("gsel", [128, 2], F32, isOutput=False)
    out_d = nc.declare_dram_parameter("out", [512, D], F16, isOutput=True)

    a2a_in = [nc.dram_tensor(f"a2a_in{h}", [1024, 512], BF16) for h in range(HPC)]
    a2a_out = [nc.dram_tensor(f"a2a_out{h}", [1024, 512], BF16) for h in range(HPC)]
    warm_in = nc.dram_tensor("warm_in", [8, 16], BF16)
    warm_out = nc.dram_tensor("warm_out", [8, 16], BF16)

    with tile.TileContext(nc) as tc:
        with tc.tile_pool(name="work", bufs=3) as wk:
            with tc.tile_pool(name="poolA", bufs=1) as pa:
                # persistent across QKV + attention
                mmul_sb = pa.tile([128, 512], BF16, tag="mmul")
                gsel_sb = pa.tile([128, 2], F32, tag="gsel")
                nc.sync.dma_start(gsel_sb[:], gsel_d[:])
                ones_mat = pa.tile([128, 128], BF16, tag="ones_mat")
                nc.vector.memset(ones_mat[:], 1.0)
                qrot = [pa.tile([128, S], BF16, tag=f"q{h}", name=f"qrot{h}") for h in range(HPC)]
                krot = [pa.tile([128, S], BF16, tag=f"k{h}", name=f"krot{h}") for h in range(HPC)]
                vsb = [pa.tile([128, 512], BF16, tag=f"v{j}", name=f"vsb{j}") for j in range(NTC)]

                # ============ Stage 1+2: QKV projections + RoPE =============
                with (
                    tc.tile_pool(name="qkvw", bufs=1) as qw,
                    tc.tile_pool(name="psq", bufs=5, space="PSUM") as psq,
                    tc.tile_pool(name="psv", bufs=2, space="PSUM") as psv,
                ):
                    xt = [qw.tile([128, S], BF16, tag=f"xt{i}", name=f"xt{i}") for i in range(NIC)]
                    wq_sb = [qw.tile([128, 512], BF16, tag=f"wq{i}", name=f"wqsb{i}") for i in range(NIC)]
                    wk_sb = [qw.tile([128, 512], BF16, tag=f"wk{i}", name=f"wksb{i}") for i in range(NIC)]
                    wv_sb = [qw.tile([128, 512], BF16, tag=f"wv{i}", name=f"wvsb{i}") for i in range(NIC)]
                    cs_sb = qw.tile([128, S], F32, tag="cs")
                    sn_sb = qw.tile([128, S], F32, tag="sn")
                    # DMA priority: x + wq stream first (gates first head's
                    # matmuls), then rope tables, then wk, wv, mask.
                    for i in range(NIC):
                        nc.sync.dma_start(xt[i][:], xT_d[128 * i : 128 * (i + 1), :])
                        nc.sync.dma_start(wq_sb[i][:], wqT_d[128 * i : 128 * (i + 1), :])
                    nc.sync.dma_start(cs_sb[:], cmat_d[:])
                    nc.sync.dma_start(sn_sb[:], smat_d[:])
                    for i in range(NIC):
                        nc.sync.dma_start(wk_sb[i][:], wkT_d[128 * i : 128 * (i + 1), :])
                    for i in range(NIC):
                        nc.sync.dma_start(wv_sb[i][:], wvT_d[128 * i : 128 * (i + 1), :])
                    nc.sync.dma_start(mmul_sb[:], mmul_d[:])

                    # Q and K projections -> transposed layout [d, tok] + RoPE.
                    # Each pass runs one head's 4 block-chains interleaved
                    # chunk-wise so every arriving x chunk feeds all of them.
                    def qk_pass(specs):
                        pss = [psq.tile([128, 512], F32, tag="qk", name=f"qk{ci}") for ci in range(len(specs))]
                        for i in range(NIC):
                            for ci, (w_sb, rot, h, b) in enumerate(specs):
                                nc.tensor.matmul(
                                    pss[ci][:],
                                    w_sb[i][:, 128 * h : 128 * (h + 1)],
                                    xt[i][:, 512 * b : 512 * (b + 1)],
                                    start=(i == 0),
                                    stop=(i == NIC - 1),
                                )
                        for ci, (w_sb, rot, h, b) in enumerate(specs):
                            ps = pss[ci]
                            shp = wk.tile([128, 512], F32, tag="shp")
                            nc.vector.stream_shuffle(shp[:], ps[:], SWAP_MASK)
                            t1 = wk.tile([128, 512], F32, tag="t1")
                            t2 = wk.tile([128, 512], F32, tag="t2")
                            nc.vector.tensor_mul(t1[:], ps[:], cs_sb[:, 512 * b : 512 * (b + 1)])
                            nc.vector.tensor_mul(t2[:], shp[:], sn_sb[:, 512 * b : 512 * (b + 1)])
                            nc.gpsimd.tensor_add(rot[h][:, 512 * b : 512 * (b + 1)], t1[:], t2[:])

                    for w_sb, rot in ((wq_sb, qrot), (wk_sb, krot)):
                        for h in range(HPC):
                            qk_pass([(w_sb, rot, h, b) for b in range(NTB)])

                    # V projection -> [tok, d] layout
                    for j in range(NTC):
                        ps = psv.tile([128, 512], F32, tag="v")
                        for i in range(NIC):
                            nc.tensor.matmul(
                                ps[:],
                                xt[i][:, 128 * j : 128 * (j + 1)],
                                wv_sb[i][:],
                                start=(i == 0),
                                stop=(i == NIC - 1),
                            )
                        nc.scalar.copy(vsb[j][:], ps[:])

                # Warm the CC path only now: any collective activity drops PE
                # matmul throughput ~18% for the rest of the kernel, so the
                # QKV phase above must run before the first collective. The
                # input copy from vsb pins the dispatch after the projections.
                nc.sync.dma_start(warm_in[:], vsb[NTC - 1][0:8, 0:16])
                nc.gpsimd.collective_compute(
                    "AllToAll",
                    mybir.AluOpType.bypass,
                    replica_groups=GROUPS,
                    ins=[warm_in[:]],
                    outs=[warm_out[:]],
                )

                with tc.tile_pool(name="wosb", bufs=1) as wop:
                    wo_sb = [wop.tile([128, D], BF16, tag=f"wo{cc}", name=f"wosb{cc}") for cc in range(NIC)]

                    # ============ Stage 3: attention per head ===============
                    with (
                        tc.tile_pool(name="attn", bufs=3) as at,
                        tc.tile_pool(name="agp", bufs=1) as agp,
                        tc.tile_pool(name="psb", bufs=3, space="PSUM") as psb,
                        tc.tile_pool(name="psav", bufs=2, space="PSUM") as psav,
                        tc.tile_pool(name="psrs", bufs=1, space="PSUM") as psrs,
                        tc.tile_pool(name="pswo", bufs=2, space="PSUM") as pswo,
                    ):
                        aglo = [agp.tile([128, 512], BF16, tag=f"aglo{g}", name=f"aglo{g}") for g in range(NIC)]
                        aghi = [agp.tile([128, 512], BF16, tag=f"aghi{g}", name=f"aghi{g}") for g in range(NIC)]
                        agc = aglo  # combined in place
                        for h in range(HPC):
                            for b in range(NTB):
                                q0 = 512 * b
                                nk2 = 4 * (b + 1)
                                av = psav.tile([128, 512], F32, tag="av")
                                rsum = psrs.tile([128, 512], F32, tag="rs")
                                for kc in range(nk2):
                                    j = kc - 4 * b  # >= 0 on the diagonal band
                                    o = 128 * j if j > 0 else 0
                                    w = 512 - o
                                    ps = psb.tile([128, 512], F32, tag="sb")
                                    nc.tensor.matmul(
                                        ps[:, :w],
                                        krot[h][:, 128 * kc : 128 * (kc + 1)],
                                        qrot[h][:, q0 + o : q0 + 512],
                                    )
                                    et = at.tile([128, 512], BF16, tag="et", bufs=6)
                                    nc.scalar.activation(et[:, :w], ps[:, :w], Exp)
                                    if j >= 0:
                                        # only the first 128 cols of a diagonal
                                        # tile contain masked elements
                                        nc.vector.tensor_mul(et[:, :128], et[:, :128], mmul_sb[:, :128])
                                    nc.tensor.matmul(
                                        rsum[:, o:512],
                                        ones_mat[:],
                                        et[:, :w],
                                        start=(kc == 0),
                                        stop=(kc == nk2 - 1),
                                    )
                                    nc.tensor.matmul(
                                        av[:, o:512],
                                        vsb[kc][:, 128 * h : 128 * (h + 1)],
                                        et[:, :w],
                                        start=(kc == 0),
                                        stop=(kc == nk2 - 1),
                                    )
                                # denominators arrive replicated across partitions
                                rbc = wk.tile([128, 512], F32, tag="rbc")
                                nc.vector.reciprocal_approx_fast(out=rbc[:], in_=rsum[:])
                                avn = at.tile([128, 512], BF16, tag="avn", bufs=6)
                                nc.vector.tensor_mul(avn[:], av[:], rbc[:])
                                # sender-side batch masking: lo slots carry data
                                # iff this core is batch 0, hi slots iff batch 1
                                avl = at.tile([128, 512], BF16, tag="avl", bufs=4)
                                avh = at.tile([128, 512], BF16, tag="avh", bufs=4)
                                nc.vector.tensor_scalar_mul(avl[:], avn[:], gsel_sb[:, 0:1])
                                nc.gpsimd.tensor_sub(avh[:], avn[:], avl[:])
                                nc.sync.dma_start(a2a_in[h][128 * b : 128 * (b + 1), :], avl[:])
                                nc.sync.dma_start(a2a_in[h][512 + 128 * b : 512 + 128 * (b + 1), :], avh[:])
                            nc.gpsimd.collective_compute(
                                "AllToAll",
                                mybir.AluOpType.bypass,
                                replica_groups=GROUPS,
                                ins=[a2a_in[h][:]],
                                outs=[a2a_out[h][:]],
                            )
                            # pull this head's chunks for all 4 peer head-groups
                            # and this head's wo rows (issued here, not up
                            # front, to keep DMA queues clear for a2a writes)
                            for r in range(4):
                                g = 4 * r + h
                                nc.sync.dma_start(aglo[g][:], a2a_out[h][128 * r : 128 * (r + 1), :])
                                nc.sync.dma_start(aghi[g][:], a2a_out[h][512 + 128 * r : 512 + 128 * (r + 1), :])
                                nc.sync.dma_start(wo_sb[g][:], woT_d[128 * g : 128 * (g + 1), :])

                        # ===== Stage 4: token-parallel wo projection ========
                        # gi order is head-major: heads 0-2 (gi 0-11) accumulate
                        # into SBUF partials while collective 3 is in flight;
                        # only the 4-matmul tail per chain needs head 3.
                        G_ORDER = [4 * r + hh for hh in range(4) for r in range(4)]
                        for g in G_ORDER:
                            nc.gpsimd.tensor_add(aglo[g][:], aglo[g][:], aghi[g][:])
                        osb = [agp.tile([128, D], F16, tag=f"osb{t}", name=f"osb{t}") for t in range(4)]
                        for t in range(4):
                            for op in range(2):
                                pss = [pswo.tile([128, 512], F32, tag="wo", name=f"wops{p}") for p in range(2)]
                                for gi in range(12):
                                    g = G_ORDER[gi]
                                    for p in range(2):
                                        oc = 2 * op + p
                                        nc.tensor.matmul(
                                            pss[p][:],
                                            agc[g][:, 128 * t : 128 * (t + 1)],
                                            wo_sb[g][:, 512 * oc : 512 * (oc + 1)],
                                            start=(gi == 0),
                                            stop=(gi == 11),
                                        )
                                for p in range(2):
                                    oc = 2 * op + p
                                    nc.scalar.copy(osb[t][:, 512 * oc : 512 * (oc + 1)], pss[p][:])
                        for t in range(4):
                            for op in range(2):
                                pss = [pswo.tile([128, 512], F32, tag="wo", name=f"wopsB{p}") for p in range(2)]
                                for gi in range(12, 16):
                                    g = G_ORDER[gi]
                                    for p in range(2):
                                        oc = 2 * op + p
                                        nc.tensor.matmul(
                                            pss[p][:],
                                            agc[g][:, 128 * t : 128 * (t + 1)],
                                            wo_sb[g][:, 512 * oc : 512 * (oc + 1)],
                                            start=(gi == 12),
                                            stop=(gi == 15),
                                        )
                                for p in range(2):
                                    oc = 2 * op + p
                                    nc.vector.tensor_add(
                                        osb[t][:, 512 * oc : 512 * (oc + 1)],
                                        pss[p][:],
                                        osb[t][:, 512 * oc : 512 * (oc + 1)],
                                    )
                            nc.sync.dma_start(out_d[128 * t : 128 * (t + 1), :], osb[t][:])

    nc.finalize()
    _GRAPH_CACHE["nc"] = nc
    return nc


def _host_prep(x, freqs_cos, freqs_sin, wq, wk, wv, wo):
    """Build the 8 per-core input maps."""
    fc = np.asarray(freqs_cos, np.float32)  # [S, 64]
    fs = np.asarray(freqs_sin, np.float32)
    cmat = np.empty((128, S), np.float32)
    smat = np.empty((128, S), np.float32)
    cmat[0::2, :] = fc.T[:, :]  # row 2i   <- cos[:, i]
    cmat[1::2, :] = fc.T[:, :]
    smat[0::2, :] = -fs.T[:, :]  # rot[2i]   = a*c - b*s ; shuf[2i]   = b
    smat[1::2, :] = fs.T[:, :]  # rot[2i+1] = b*c + a*s ; shuf[2i+1] = a

    xs = np.arange(128)[:, None]
    ys = np.arange(512)[None, :]
    # AV-path mask for [128 k x 512 q] diagonal tiles: valid iff x <= y
    mmul = (xs <= ys).astype(np.float32)

    wq_s = np.asarray(wq, np.float32) / math.sqrt(DH)
    wk_s = np.asarray(wk, np.float32)
    wv_s = np.asarray(wv, np.float32)
    woT = np.ascontiguousarray(np.asarray(wo, np.float32).T).astype(bf16)
    x = np.asarray(x, np.float32)

    shared = {
        "cmat": cmat,
        "smat": smat,
        "mmul": mmul.astype(bf16),
        "woT": woT,
    }
    in_maps = []
    for c in range(8):
        b, g = c // 4, c % 4
        hs = slice(512 * g, 512 * (g + 1))
        m = dict(shared)
        m["xT"] = np.ascontiguousarray(x[b].T).astype(bf16)
        m["wqT"] = np.ascontiguousarray(wq_s[hs, :].T).astype(bf16)
        m["wkT"] = np.ascontiguousarray(wk_s[hs, :].T).astype(bf16)
        m["wvT"] = np.ascontiguousarray(wv_s[hs, :].T).astype(bf16)
        gsel = np.zeros((128, 2), np.float32)
        gsel[:, b] = 1.0
        m["gsel"] = gsel
        in_maps.append(m)
    return in_maps


def kernel(x, freqs_cos, freqs_sin, mask, wq, wk, wv, wo):
    in_maps = _host_prep(x, freqs_cos, freqs_sin, wq, wk, wv, wo)
    nc = build_graph()
    results = run_bass_kernel_spmd(nc, in_maps, core_ids=list(range(8))).results
    out = np.empty((B, S, D), np.float32)
    for c in range(8):
        b, g = c // 4, c % 4
        out[b, 512 * g : 512 * (g + 1), :] = results[c]["out"]
    return out
